# revision 1
# baseline (speedup 1.0000x reference)
"""CrossAgentAttention Trainium2 kernel (bf16).

Problem: B=1024 samples, N=32 agents, D=512 features, H=4 heads (HD=128).
  qkv = x @ Win^T + bin ; per-head attention over the N=32 agents with the
  diagonal (self) and padded agents masked out of the keys; out = ctx @ Wout^T + bout.

Strategy (data-parallel over B across 8 cores, weights replicated):
  - Host pre-transposes the per-core activations to X^T [D, T] (T = B/8*32
    tokens) and the weights to Win^T / Wout^T so every GEMM contraction dim
    lands on SBUF partitions.  Q columns of Win^T are pre-scaled by
    1/sqrt(HD).  Everything is cast to bf16 on host (matmul moving operands
    at 128-wide free dims run 4x faster in bf16 than f32r, and DMA halves).
  - Stage 1: Q^T,K^T [1024, T] in transposed (feature-major) layout and
    V [T, 512] token-major, via bf16 matmuls with N=512 moving operands.
  - Stage 2: attention per (sample-group of 4, head).  128 tokens = 4 samples
    x 32 agents: S = Q^T.T @ K^T gives all 16 cross-sample blocks.  exp() runs
    directly on the PSUM scores (no max-subtraction; logits are O(1) by
    construction, junk blocks are finite).  A multiplicative {0,1} mask kills
    cross-sample blocks, the self-diagonal, and padded keys, fused with the
    per-head row-sums in one DVE scalar_tensor_tensor pass per head.  One
    broadcast multiply normalizes.  P^T via a single DVE stream-transpose
    (P's cross-sample 32x32 blocks are exactly zero, so the block-local
    transpose equals the full one), then ctx^T = V^T @ P^T on PE.
  - Stage 3: OUT^T = Wout^T.T @ ctx^T, DMA out bf16; host transposes back.
  - Schedule: two-group-deep software pipeline (group g's stage-1 GEMMs hide
    group g-1's softmax latency; out-proj of g-2 fills the remaining PE
    window), input prefetch one group ahead on the SP DMA queue, output
    writeback on the idle GPSIMD queue, and 8 reps unrolled per For_i
    iteration to amortize the hardware loop's all-engine barrier.
"""

import math

import numpy as np
import ml_dtypes

import concourse.bass as bass
import concourse.mybir as mybir
import concourse.tile as tile
from concourse import bacc
from concourse.bass_utils import run_bass_kernel_spmd

N_CORES = 8
B, N, D, H = 1024, 32, 512, 4
HD = D // H  # 128
F32 = mybir.dt.float32
BF16 = mybir.dt.bfloat16
NPBF16 = ml_dtypes.bfloat16


def build_program(b_core, reps=1, with_pad=False, with_bias=False, unroll=False):
    """Trace + compile the per-core program. Returns nc."""
    T = b_core * N  # tokens per core
    GT = 512 if T >= 512 else T  # tokens per group
    G = T // GT  # groups
    TT = GT // 128  # 128-token tiles (sample groups of 4) per group
    assert T % 512 == 0 or G == 1

    nc = bacc.Bacc("TRN2", target_bir_lowering=False, debug=False, num_devices=N_CORES)

    MD = BF16  # matmul-operand dtype
    xt = nc.dram_tensor("xt", [D, T], MD, kind="ExternalInput").ap()
    wint = nc.dram_tensor("wint", [D, 3 * D], MD, kind="ExternalInput").ap()
    woutt = nc.dram_tensor("woutt", [D, D], MD, kind="ExternalInput").ap()
    # binary {0,1} keep-mask; head-independent ([q=(s,i), k=(s',j)] pattern)
    if with_pad:
        mask = nc.dram_tensor("mask", [T // 128, 128, 128], MD,
                              kind="ExternalInput").ap()
    else:
        mask = nc.dram_tensor("mask", [128, 128], MD, kind="ExternalInput").ap()
    if with_bias:
        bqk = nc.dram_tensor("bqk", [128, 8], F32, kind="ExternalInput").ap()
        bv = nc.dram_tensor("bv", [1, D], MD, kind="ExternalInput").ap()
        bo = nc.dram_tensor("bo", [128, 4], F32, kind="ExternalInput").ap()
    outt = nc.dram_tensor("outt", [D, T], MD, kind="ExternalOutput").ap()

    with tile.TileContext(nc) as tc:
        with (
            tc.tile_pool(name="wpool", bufs=1) as wpool,
            tc.tile_pool(name="xtp", bufs=2 * 4, space="SBUF") as xtp,
            tc.tile_pool(name="qktp", bufs=2 * 8) as qktp,
            tc.tile_pool(name="vp", bufs=2 * TT) as vp,
            tc.tile_pool(name="smp", bufs=4) as smp,
            tc.tile_pool(name="ctxp", bufs=2) as ctxp,
            tc.tile_pool(name="otp", bufs=4) as otp,
            tc.tile_pool(name="mmps", bufs=2, space="PSUM") as mmps,
            tc.tile_pool(name="opps", bufs=2, space="PSUM") as opps,
            tc.tile_pool(name="spsp", bufs=2, space="PSUM") as spsp,
            tc.tile_pool(name="tpsp", bufs=2, space="PSUM") as tpsp,
        ):
            # ---- resident weights / constants ----
            w = []
            for k in range(4):
                wt = wpool.tile([128, 3 * D], MD, tag=f"wint{k}")
                w.append(wt)
            # chunked so Q columns (chunk 0) land first; K then V follow
            for c in range(3):
                for k in range(4):
                    nc.sync.dma_start(
                        w[k][:, bass.ts(c, D)],
                        wint[k * 128:(k + 1) * 128, bass.ts(c, D)])
            mk_const = None
            if not with_pad:
                mk_const = wpool.tile([128, 128], MD, tag="mask")
                nc.sync.dma_start(mk_const[:], mask[:])
            wo = []
            for k in range(4):
                wt = wpool.tile([128, D], MD, tag=f"woutt{k}")
                nc.sync.dma_start(wt[:], woutt[k * 128:(k + 1) * 128, :])
                wo.append(wt)
            if with_bias:
                bqk_sb = wpool.tile([128, 8], F32, tag="bqk")
                nc.sync.dma_start(bqk_sb[:], bqk[:])
                bv_sb = wpool.tile([1, D], MD, tag="bv")
                nc.sync.dma_start(bv_sb[:], bv[:])
                bo_sb = wpool.tile([128, 4], F32, tag="bo")
                nc.sync.dma_start(bo_sb[:], bo[:])
                ones_sb = wpool.tile([1, 128], MD, tag="ones")
                nc.vector.memset(ones_sb[:], 1.0)

            def body(_iv=None):
                # Two-group-deep software pipeline: during group g's stage-1
                # GEMMs, group g-1's softmax (long ACT/DVE latency chain) runs
                # in the shadow, and out-proj of group g-2 fills the remaining
                # PE window before g-1's transposes/ctx matmuls.
                xgs, qkts, vgs, ctxts = {}, {}, {}, {}
                pnbs, ptsbs = {}, {}

                def load_xg(g):
                    if g >= G:
                        return
                    xg = []
                    for k in range(4):
                        t = xtp.tile([128, GT], MD, tag="xt")
                        nc.sync.dma_start(
                            t[:], xt[k * 128:(k + 1) * 128, bass.ts(g, GT)])
                        xg.append(t)
                    xgs[g] = xg

                def stage1a(g, fos):
                    xg = xgs[g]
                    qkt = qkts.setdefault(g, {})
                    for fo in fos:
                        ps = mmps.tile([128, GT], F32, tag="mm")
                        for k in range(4):
                            nc.tensor.matmul(
                                ps[:],
                                w[k][:, bass.ts(fo, 128)],
                                xg[k][:],
                                start=(k == 0), stop=(k == 3),
                            )
                        qt = qktp.tile([128, GT], MD, tag="qkt")
                        if with_bias:
                            nc.scalar.activation(
                                qt[:], ps[:], mybir.ActivationFunctionType.Identity,
                                bias=bqk_sb[:, fo:fo + 1])
                        else:
                            nc.scalar.copy(qt[:], ps[:])
                        qkt[fo] = qt

                def stage1b(g):
                    xg = xgs.pop(g)
                    vg = []
                    for tt in range(TT):
                        ps = mmps.tile([128, D], F32, tag="mm")
                        for k in range(4):
                            nc.tensor.matmul(
                                ps[:],
                                xg[k][:, bass.ts(tt, 128)],
                                w[k][:, 2 * D:3 * D],
                                start=(k == 0), stop=(k == 3 and not with_bias),
                            )
                        if with_bias:
                            nc.tensor.matmul(
                                ps[:], ones_sb[:],
                                bv_sb[:],
                                start=False, stop=True,
                            )
                        vt = vp.tile([128, D], MD, tag="v")
                        nc.vector.tensor_copy(vt[:], ps[:])
                        vg.append(vt)
                    vgs[g] = vg

                def outproj(g, half=None):
                    # half=0/1 emits only that token-half (256 cols) so the
                    # pipeline tail can interleave with the last ctx matmuls
                    ctxt_prev = ctxts[g] if half == 0 else ctxts.pop(g)
                    HT = GT if half is None else GT // 2
                    base = 0 if half in (None, 0) else GT // 2
                    for fo in range(4):
                        ps = opps.tile([128, GT], F32, tag="op")
                        for k in range(4):
                            nc.tensor.matmul(
                                ps[:, 0:HT],
                                wo[k][:, bass.ts(fo, 128)],
                                ctxt_prev[:, k, base:base + HT],
                                start=(k == 0), stop=(k == 3),
                            )
                        ot = otp.tile([128, GT], MD, tag="ot")
                        if with_bias:
                            nc.scalar.activation(
                                ot[:, 0:HT], ps[:, 0:HT],
                                mybir.ActivationFunctionType.Identity,
                                bias=bo_sb[:, fo:fo + 1])
                        else:
                            nc.scalar.copy(ot[:, 0:HT], ps[:, 0:HT])
                        nc.gpsimd.dma_start(
                            outt[fo * 128:(fo + 1) * 128,
                                 g * GT + base:g * GT + base + HT], ot[:, 0:HT])

                def stA(g, tt):
                    qkt = qkts[g]
                    ttsl = bass.ts(tt, 128)
                    if with_pad:
                        mk = smp.tile([128, 128], MD, tag="mask")
                        nc.sync.dma_start(mk[:], mask[g * TT + tt])
                    else:
                        mk = mk_const
                    sps = spsp.tile([128, 4 * 128], F32, tag="sps")
                    for h in range(4):
                        nc.tensor.matmul(
                            sps[:, bass.ts(h, 128)],
                            qkt[h][:, ttsl],
                            qkt[4 + h][:, ttsl],
                            start=True, stop=True,
                        )
                    # exp of raw scores straight out of PSUM (junk blocks
                    # stay finite; the {0,1} mask zeroes them next)
                    psb = smp.tile([128, 4 * 128], MD, tag="psb")
                    nc.scalar.activation(
                        psb[:], sps[:], mybir.ActivationFunctionType.Exp)
                    # masked P and per-head row-sums in one DVE pass/head
                    pnm = smp.tile([128, 4 * 128], MD, tag="pnm")
                    rsum = smp.tile([128, 8], F32, tag="rsum")
                    for h in range(4):
                        nc.vector.scalar_tensor_tensor(
                            pnm[:, bass.ts(h, 128)],
                            psb[:, bass.ts(h, 128)],
                            0.0,
                            mk[:],
                            mybir.AluOpType.bypass,
                            mybir.AluOpType.mult,
                            accum_out=rsum[:, h:h + 1],
                        )
                    nc.vector.reciprocal(rsum[:, 4:8], rsum[:, 0:4])
                    pnb = smp.tile([128, 4 * 128], MD, tag="pnb")
                    rb = rsum[:, 4:8]
                    rinv_b = bass.AP(tensor=rb.tensor, offset=rb.offset,
                                     ap=list(rb.ap) + [[0, 128]])
                    nc.vector.tensor_mul(
                        pnb[:].rearrange("p (h j) -> p h j", h=4),
                        pnm[:].rearrange("p (h j) -> p h j", h=4),
                        rinv_b)
                    pnbs[(g, tt)] = pnb

                def stB(g, tt):
                    # P's cross-sample 32x32 blocks are exactly zero, so the
                    # full per-head 128x128 transpose equals a block-local
                    # 32x32 transpose: one DVE stream-transpose, no PE, no
                    # PSUM round-trip.
                    pnb = pnbs.pop((g, tt))
                    ptsb = smp.tile([128, 4 * 128], MD, tag="ptsb")
                    nc.vector.transpose(ptsb[:], pnb[:])
                    ptsbs[(g, tt)] = ptsb

                def stC(g, tt):
                    ttsl = bass.ts(tt, 128)
                    ptsb = ptsbs.pop((g, tt))
                    if tt == 0:
                        ctxt = ctxp.tile([128, 4, GT], MD, tag="ctxt")
                        ctxts[g] = ctxt
                    ctxt = ctxts[g]
                    cps = tpsp.tile([128, 4 * 128], F32, tag="tp")
                    for h in range(4):
                        nc.tensor.matmul(
                            cps[:, bass.ts(h, 128)],
                            vgs[g][tt][:, bass.ts(h, 128)],
                            ptsb[:, bass.ts(h, 128)],
                            start=True, stop=True,
                        )
                    nc.scalar.copy(
                        ctxt[:, :, ttsl],
                        cps[:].rearrange("p (h q) -> p h q", h=4))

                def halves(n):
                    cut = min(2, n)
                    return range(cut), range(cut, n)

                load_xg(0)
                for g in range(G):
                    p, q = g - 1, g - 2
                    tt_lo, tt_hi = halves(TT)
                    load_xg(g + 1)
                    if p >= 0:
                        for t in tt_lo:
                            stA(p, t)
                    stage1a(g, range(0, 4))
                    if p >= 0:
                        for t in tt_hi:
                            stA(p, t)
                        for t in tt_lo:
                            stB(p, t)
                    stage1a(g, range(4, 8))
                    if p >= 0:
                        for t in tt_hi:
                            stB(p, t)
                    stage1b(g)
                    if q >= 0:
                        outproj(q)
                    if p >= 0:
                        for t in range(TT):
                            stC(p, t)
                    vgs.pop(p, None)
                # pipeline tail: last group's attention + last two out-projs;
                # the final out-proj is emitted in token-halves so its GEMMs
                # interleave with the last ctx matmuls instead of waiting for
                # every ctx copy.
                p = G - 1
                tt_lo, tt_hi = halves(TT)
                for t in tt_lo:
                    stA(p, t)
                for t in tt_hi:
                    stA(p, t)
                for t in tt_lo:
                    stB(p, t)
                if G >= 2:
                    outproj(G - 2)
                for t in tt_hi:
                    stB(p, t)
                for t in range(TT):
                    stC(p, t)
                outproj(p)

            if reps == 1:
                body()
            elif unroll:
                for _ in range(reps):
                    body()
            else:
                # unroll several reps inside each For_i iteration: the
                # hardware loop's all-engine barrier drains the software
                # pipeline, so amortize it over UF reps
                UF = 8 if reps % 8 == 0 else 1
                with tc.For_i(0, reps // UF, 1, hint_engines=(
                        mybir.EngineType.PE, mybir.EngineType.DVE,
                        mybir.EngineType.Activation, mybir.EngineType.SP)) as iv:
                    for _ in range(UF):
                        body(iv)

    nc.compile()
    return nc


def make_host_inputs(agent_hiddens, padding_mask, in_proj_weight, in_proj_bias,
                     out_proj_weight, out_proj_bias):
    """Shard + lay out host-side numpy arrays. Returns (in_maps, flags)."""
    x = np.asarray(agent_hiddens, dtype=np.float32)
    pad = np.asarray(padding_mask)
    win = np.asarray(in_proj_weight, dtype=np.float32)
    bin_ = np.asarray(in_proj_bias, dtype=np.float32)
    wout = np.asarray(out_proj_weight, dtype=np.float32)
    bout = np.asarray(out_proj_bias, dtype=np.float32)

    b = x.shape[0]
    b_core = b // N_CORES
    T = b_core * N
    scale = 1.0 / math.sqrt(HD)

    with_pad = bool(pad.any())
    with_bias = bool(bin_.any() or bout.any())

    wint = np.ascontiguousarray(win.T)
    wint[:, :D] *= scale
    woutt = np.ascontiguousarray(wout.T)

    # 128-token block keep-mask {0,1}: tokens (s, i) x (s', j); kill
    # cross-sample blocks and the global diagonal (self-attention).
    p = np.arange(128)
    blockmask = np.where((p[:, None] // 32 != p[None, :] // 32)
                         | (p[:, None] == p[None, :]), 0.0, 1.0).astype(np.float32)

    in_maps = []
    for c in range(N_CORES):
        xc = x[c * b_core:(c + 1) * b_core].reshape(T, D)
        m = {
            "xt": np.ascontiguousarray(xc.T).astype(NPBF16),
            "wint": wint.astype(NPBF16),
            "woutt": woutt.astype(NPBF16),
        }
        if with_pad:
            padc = pad[c * b_core:(c + 1) * b_core]  # [b_core, N]
            n_tt = T // 128
            mt = np.empty((n_tt, 128, 128), dtype=np.float32)
            for t in range(n_tt):
                # 4 samples in this tile; key-padding kills columns
                pr = padc[t * 4:(t + 1) * 4].reshape(128)  # [(s', j)] order
                mt[t] = blockmask * np.where(pr[None, :], 0.0, 1.0)
            m["mask"] = mt.astype(NPBF16)
        else:
            m["mask"] = blockmask.astype(NPBF16)
        if with_bias:
            bq = bin_[:D] * scale
            bk = bin_[D:2 * D]
            m["bqk"] = np.ascontiguousarray(
                np.concatenate([bq, bk]).reshape(8, 128).T)
            m["bv"] = bin_[2 * D:3 * D].reshape(1, D).astype(NPBF16)
            m["bo"] = np.ascontiguousarray(bout.reshape(4, 128).T)
        in_maps.append(m)
    return in_maps, dict(b_core=b_core, with_pad=with_pad, with_bias=with_bias)


def assemble_output(results, b_core):
    outs = []
    for c in range(N_CORES):
        ot = np.asarray(results[c]["outt"], dtype=np.float32)  # [D, T]
        outs.append(ot.T.reshape(b_core, N, D))
    return np.ascontiguousarray(np.concatenate(outs, axis=0))


_NC_CACHE = {}


def _get_nc(key_args):
    key = tuple(sorted(key_args.items()))
    if key not in _NC_CACHE:
        _NC_CACHE[key] = build_program(**key_args)
    return _NC_CACHE[key]


def kernel(agent_hiddens, padding_mask, in_proj_weight, in_proj_bias,
           out_proj_weight, out_proj_bias):
    in_maps, meta = make_host_inputs(
        agent_hiddens, padding_mask, in_proj_weight, in_proj_bias,
        out_proj_weight, out_proj_bias)
    nc = _get_nc(dict(b_core=meta["b_core"], reps=1,
                      with_pad=meta["with_pad"], with_bias=meta["with_bias"]))
    res = run_bass_kernel_spmd(nc, in_maps, list(range(N_CORES)))
    return assemble_output(res.results, meta["b_core"])

